# revision 19
# baseline (speedup 1.0000x reference)
"""Trainium2 Bass kernel for the Mamba U-Net model (nn_Model_20770461843918).

Batch-data-parallel SPMD over 8 NeuronCores (4 batch elements; cores c and
c+4 duplicate work, outputs read from cores 0-3).  Per core the whole
7-block Mamba U-Net runs locally with partitions = inner channel d:
  PE  : all matmuls (in/x/dt/out projections, depthwise conv via diagonal
        matmuls, down/up/gate convs), plus the n-state reduction
        y = sum_n h*C as accumulating identity matmuls into PSUM
  DMA : per-timestep B/C rows broadcast across the 128 partitions straight
        from the DRAM bounce buffer (partition-stride-0 descriptors)
  ACT : exp(dt*A) per state n, exp/log for softplus and for sigmoid
        (sigmoid = 1/(1+exp(-x)) so every activation stays in the single
        natural_log_exp act table -> one table load total)
  DVE : selective scan via tensor_tensor_scan (internal fp32 state, bf16
        operands/output), fast reciprocal, share of elementwise muls
  Pool: remaining share of the dBu/h*C elementwise muls
Precision split: the U-Net trunk (level tensors, in/out projections, gates,
downconvs) stays fp32; the scan-stage inner pipeline (conv->u, x-proj, dt,
dBu/h/h*C cubes, B/C reps) rides in bf16 with fp32 scan state and PSUM.
"""
import numpy as np
import ml_dtypes

B, L0, C = 4, 1024, 128
DI, NST, R, KC = 256, 16, 8, 4
NV = NST + 4          # packed per-partition vec cols: A[16], D, convb, bdt, -convb
NCORES = 8
TS = 512              # scan-stage time chunk
MM = 512              # matmul-stage time chunk


# bf16 panel pack layout (cols): identity only
TOTW16 = 128
# f32 panel pack layout (cols): per block wxT(2x64)+woutT (2x128), per gate
# dcwT(384)+upw(256)+wgT(256)+dbT(256), then 7x2xNV vec cols + 3x5 gate vecs
BLKW32 = 128 + 256
GATW32 = 384 + 256 + 256 + 256
TOTW32 = 7 * BLKW32 + 3 * GATW32 + 7 * 2 * NV + 3 * 5

_CACHE = {}


def _prep_weights(inp):
    f32 = np.float32
    bf16 = ml_dtypes.bfloat16
    g = lambda k: np.asarray(inp[k], f32)
    m_Win, m_convw, m_convb = g("m_Win"), g("m_convw"), g("m_convb")
    m_Wx, m_Wdt, m_bdt = g("m_Wx"), g("m_Wdt"), g("m_bdt")
    m_Alog, m_D, m_Wout = g("m_Alog"), g("m_D"), g("m_Wout")
    dc_w, dc_b = g("dc_w"), g("dc_b")
    wg_W, wg_b, db_W, db_b = g("wg_W"), g("wg_b"), g("db_W"), g("db_b")
    up_w, up_b = g("up_w"), g("up_b")

    winT = np.ascontiguousarray(m_Win.transpose(0, 2, 1))                # [7, C, 512] f32
    cd = np.zeros((7, 2, KC, 128, 128), f32)
    idx = np.arange(128)
    for i in range(7):
        for gg in range(2):
            for k in range(KC):
                cd[i, gg, k, idx, idx] = m_convw[i, gg * 128:(gg + 1) * 128, k]
    # sbuf layout [128, (g, k, 128)]: partition = k_in, free-block (g,k) = lhsT
    convdiag = np.ascontiguousarray(
        cd.transpose(0, 1, 3, 2, 4)).reshape(7, 2, 128, KC * 128)
    wxT_raw = np.ascontiguousarray(m_Wx.transpose(0, 2, 1)).reshape(7, 2, 128, R + 2 * NST)
    wxT = np.zeros((7, 2, 128, 64), f32)
    wxT[..., :R] = wxT_raw[..., :R]          # dt rows -> psum partitions 0..7
    wxT[..., 32:64] = wxT_raw[..., R:]       # B/C rows -> psum partitions 32..63
    wdtT = np.ascontiguousarray(m_Wdt.transpose(0, 2, 1))                # [7, R, DI]
    wdtall = wdtT.transpose(1, 0, 2).reshape(R, 7 * DI).astype(bf16)
    A = -np.exp(m_Alog)                                                  # [7, DI, N]
    vec = np.zeros((7, 2, 128, NV), f32)
    for gg in range(2):
        sl = slice(gg * 128, (gg + 1) * 128)
        vec[:, gg, :, :NST] = A[:, sl, :]
        vec[:, gg, :, NST] = m_D[:, sl]
        vec[:, gg, :, NST + 1] = m_convb[:, sl]
        vec[:, gg, :, NST + 2] = m_bdt[:, sl]
        vec[:, gg, :, NST + 3] = -m_convb[:, sl]
    woutT = np.ascontiguousarray(m_Wout.transpose(0, 2, 1)).reshape(7, 2, 128, C)
    # dc_w[j, co, ci, k] -> [j, ci, (k, co)]
    dcwT = np.ascontiguousarray(dc_w.transpose(0, 2, 3, 1)).reshape(3, 128, 3 * 128)
    # up_w[j, ci, co, k] -> [j, ci, (k, co)]
    upw = np.ascontiguousarray(up_w.transpose(0, 1, 3, 2)).reshape(3, 128, 2 * 128)
    wgT = np.ascontiguousarray(wg_W.transpose(0, 2, 1)).reshape(3, 2, 128, 128)
    dbT = np.ascontiguousarray(db_W.transpose(0, 2, 1)).reshape(3, 2, 128, 128)
    gv = np.zeros((3, 128, 5), f32)
    gv[:, :, 0], gv[:, :, 1], gv[:, :, 2], gv[:, :, 3] = dc_b, up_b, wg_b, db_b
    gv[:, :, 4] = -wg_b
    # bf16 pack: identity only
    p16 = [np.eye(128, dtype=bf16)]
    # f32 pack: wxT, woutT, gate weights, vec cols
    p32 = []
    for i in range(7):
        p32 += [wxT[i, 0], wxT[i, 1], woutT[i, 0], woutT[i, 1]]
    for j in range(3):
        p32 += [dcwT[j], upw[j], wgT[j, 0], wgT[j, 1], dbT[j, 0], dbT[j, 1]]
    p32 += [vec[i, gg] for i in range(7) for gg in range(2)]
    p32 += [gv[j] for j in range(3)]
    return {"winT": winT, "convdiag": convdiag, "wdtall": wdtall,
            "wtpack16": np.ascontiguousarray(np.concatenate(p16, axis=1)),
            "wtpack32": np.ascontiguousarray(np.concatenate(p32, axis=1))}


def _build():
    import concourse.bacc as bacc
    import concourse.tile as tile
    import concourse.mybir as mybir
    from concourse.hw_specs import get_activation_tables as _gat

    F32 = mybir.dt.float32
    BF16 = mybir.dt.bfloat16
    Alu = mybir.AluOpType
    Act = mybir.ActivationFunctionType

    # Steer the act-table placement pass to the one table that holds every
    # function we use (Copy/Identity/Exp/Ln), so exactly one table load is
    # emitted.  Table ids stay aligned with act_info.json: we only hide our
    # functions from the OTHER tables during placement.
    _KEEP = "natural_log_exp_and_others"
    _OURS = {Act.Copy, Act.Identity, Act.Exp, Act.Ln}
    _orig_gat = bacc.get_activation_tables

    def _patched_gat(arch):
        tabs = _gat(arch)
        assert _KEEP in tabs and _OURS <= tabs[_KEEP]
        return {k: (v if k == _KEEP else set(v) - _OURS) for k, v in tabs.items()}

    nc = bacc.Bacc("TRN2", target_bir_lowering=False, debug=False,
                   num_devices=NCORES)

    xT_d = nc.declare_dram_parameter("xT", [C, L0], F32, isOutput=False)
    out_d = nc.declare_dram_parameter("out", [C, L0], F32, isOutput=True)
    dram = {}
    for name, shape, dt in [
        ("winT", [7, C, 2 * DI], F32),
        ("convdiag", [7, 2, 128, KC * 128], F32),
        ("wdtall", [R, 7 * DI], BF16),
        ("wtpack16", [128, TOTW16], BF16),
        ("wtpack32", [128, TOTW32], F32),
    ]:
        dram[name] = nc.declare_dram_parameter(name, shape, dt, isOutput=False)
    bc_dram4 = [[nc.dram_tensor(f"bc_bounce{p}_{s}", [2 * NST, TS], BF16)
                 for s in range(2)] for p in range(2)]

    with tile.TileContext(nc) as tc:
        with tc.tile_pool(name="wt", bufs=1) as wt, \
             tc.tile_pool(name="lvl", bufs=1) as lvl, \
             tc.tile_pool(name="blk", bufs=1) as blk, \
             tc.tile_pool(name="cube", bufs=2) as cube, \
             tc.tile_pool(name="repk", bufs=2) as repk, \
             tc.tile_pool(name="cw", bufs=2) as cw, \
             tc.tile_pool(name="ubuf", bufs=1) as ubuf, \
             tc.tile_pool(name="gw", bufs=1) as gw, \
             tc.tile_pool(name="cwc", bufs=2) as cwc, \
             tc.tile_pool(name="mmp", bufs=3, space="PSUM") as mmp, \
             tc.tile_pool(name="yp", bufs=4, space="PSUM") as yp, \
             tc.tile_pool(name="xdbp", bufs=1, space="PSUM") as xdbp:

            def load_blk(i):
                winTb = cw.tile([C, 2 * DI], F32, tag="winT", name=f"winTb{i}")
                nc.scalar.dma_start(winTb[:], dram["winT"][i])
                cdw = cwc.tile([128, 2 * KC * 128], F32, tag="convdiag",
                               name=f"cdw{i}")
                nc.scalar.dma_start(cdw[:, :KC * 128], dram["convdiag"][i, 0])
                nc.scalar.dma_start(cdw[:, KC * 128:], dram["convdiag"][i, 1])
                return cdw, winTb

            preload = {0: load_blk(0)}

            w16 = wt.tile([128, TOTW16], BF16, tag="w16")
            nc.scalar.dma_start(w16[:], dram["wtpack16"][:])
            w32 = wt.tile([128, TOTW32], F32, tag="w32")
            nc.scalar.dma_start(w32[:, :TOTW32 // 2], dram["wtpack32"][:, :TOTW32 // 2])
            nc.scalar.dma_start(w32[:, TOTW32 // 2:], dram["wtpack32"][:, TOTW32 // 2:])
            wdtall = wt.tile([R, 7 * DI], BF16, tag="wdtall")
            nc.scalar.dma_start(wdtall[:], dram["wdtall"][:])
            ident = w16[:, :]
            wdtTt = [wdtall[:, i * DI:(i + 1) * DI] for i in range(7)]
            wxTt = [w32[:, i * BLKW32:i * BLKW32 + 128] for i in range(7)]
            woutTt = [w32[:, i * BLKW32 + 128:(i + 1) * BLKW32] for i in range(7)]
            og = 7 * BLKW32
            dcwTt, upwt, wgTt, dbTt = [], [], [], []
            for j in range(3):
                o = og + j * GATW32
                dcwTt.append(w32[:, o:o + 384])
                upwt.append(w32[:, o + 384:o + 640])
                wgTt.append(w32[:, o + 640:o + 896])
                dbTt.append(w32[:, o + 896:o + 1152])
            ov = og + 3 * GATW32
            vecst = [w32[:, ov + i * 2 * NV:ov + (i + 1) * 2 * NV] for i in range(7)]
            ogv = ov + 14 * NV
            gvecst = [w32[:, ogv + j * 5:ogv + j * 5 + 5] for j in range(3)]

            # per-block working tiles (reused across blocks)
            xi = [blk.tile([128, L0 + 3], F32, tag=f"xi{g}", name=f"xi{g}")
                  for g in range(2)]
            y_t = [blk.tile([128, L0], F32, tag=f"y{g}", name=f"y{g}")
                   for g in range(2)]
            xdbR = blk.tile([R, L0], BF16, tag="xdbR")
            bc16 = blk.tile([2 * NST, L0], BF16, tag="bc16")
            carry = blk.tile([128, 2 * NST], BF16, tag="carry")

            def mamba(x_ap, i, Lb, out_ap, out_dma=None):
                cdw, winTb = preload.pop(i) if i in preload else load_blk(i)
                u_t = [ubuf.tile([128, L0], F32, tag=f"u{g}", name=f"u{g}_{i}")
                       for g in range(2)]
                dt_t = [ubuf.tile([128, L0], BF16, tag=f"dt{g}", name=f"dt{g}_{i}")
                        for g in range(2)]
                vecs = vecst[i]
                ypss = {}

                def vcol(g, c):
                    return vecs[:, g * NV + c: g * NV + c + 1]
                # ---- stage M ----
                for c0 in range(0, Lb, MM):
                    F = min(MM, Lb - c0)
                    ztmp = cw.tile([128, MM], F32, tag="ztmp", name="ztmpM")
                    for p in range(2):
                        ps = mmp.tile([128, MM], F32, tag="mmps")
                        nc.tensor.matmul(ps[:, :F], winTb[:, p * 128:(p + 1) * 128],
                                         x_ap[:, c0:c0 + F], start=True, stop=True)
                        nc.scalar.activation(xi[p][:, 3 + c0:3 + c0 + F], ps[:, :F], Act.Copy)
                    for g in range(2):
                        ps = mmp.tile([128, MM], F32, tag="mmps")
                        for k in range(KC):
                            nc.tensor.matmul(
                                ps[:, :F],
                                cdw[:, (g * KC + k) * 128:(g * KC + k + 1) * 128],
                                xi[g][:, c0 + k:c0 + k + F],
                                start=(k == 0), stop=(k == KC - 1))
                        nc.scalar.activation(u_t[g][:, c0:c0 + F], ps[:, :F], Act.Identity,
                                             bias=vcol(g, NST + 1))
                        # sigmoid via exp (stay in the exp/ln act table):
                        # e = exp(-(x+b)); u = (x+b) / (1+e)
                        nc.scalar.activation(ztmp[:, :F], ps[:, :F], Act.Exp,
                                             bias=vcol(g, NST + 3), scale=-1.0)
                        nc.vector.tensor_scalar_add(ztmp[:, :F], ztmp[:, :F], 1.0)
                        nc.vector.reciprocal_approx_fast(ztmp[:, :F], ztmp[:, :F])
                        nc.vector.tensor_mul(u_t[g][:, c0:c0 + F], u_t[g][:, c0:c0 + F],
                                             ztmp[:, :F])
                    psx = xdbp.tile([64, MM], F32, tag="xdbps")
                    for g in range(2):
                        nc.tensor.matmul(psx[:, :F],
                                         wxTt[i][:, g * 64:(g + 1) * 64],
                                         u_t[g][:, c0:c0 + F], start=(g == 0), stop=(g == 1))
                    nc.scalar.activation(xdbR[:, c0:c0 + F], psx[:R, :F], Act.Copy)
                    nc.scalar.activation(bc16[:, c0:c0 + F], psx[32:, :F], Act.Copy)
                    nc.sync.dma_start(bc_dram4[i % 2][c0 // TS][:, :F],
                                      bc16[:, c0:c0 + F])
                    for g in range(2):
                        ps = mmp.tile([128, MM], F32, tag="mmps")
                        nc.tensor.matmul(ps[:, :F], wdtTt[i][:, g * 128:(g + 1) * 128],
                                         xdbR[:, c0:c0 + F], start=True, stop=True)
                        nc.scalar.activation(ztmp[:, :F], ps[:, :F], Act.Exp,
                                             bias=vcol(g, NST + 2))
                        nc.scalar.activation(dt_t[g][:, c0:c0 + F], ztmp[:, :F], Act.Ln,
                                             bias=1.0)
                # ---- stage S ----
                nchunks = (Lb + TS - 1) // TS
                for s in range(nchunks):
                    s0 = s * TS
                    F = min(TS, Lb - s0)
                    bc_dram = bc_dram4[i % 2][s]
                    # B/C rows broadcast across partitions straight from DRAM
                    repB = repk.tile([128, NST * TS], BF16, tag="rep", name="repB")
                    repC = repk.tile([128, NST * TS], BF16, tag="rep", name="repC")
                    for hq in range(2):
                        nh = 8 * hq
                        nc.sync.dma_start(
                            repB[:, nh * F:(nh + 8) * F].rearrange(
                                "p (n t) -> p n t", n=8),
                            bc_dram[nh:nh + 8, :F].unsqueeze(0).broadcast_to(
                                [128, 8, F]))
                    nc.sync.dma_start(
                        repC[:, :NST * F].rearrange("p (n t) -> p n t", n=NST),
                        bc_dram[NST:, :F].unsqueeze(0).broadcast_to([128, NST, F]))
                    for g in range(2):
                        dA_t = cube.tile([128, NST * TS], BF16, tag="dA")
                        dBu_t = cube.tile([128, NST * TS], BF16, tag="dBu")
                        dtu = cw.tile([128, TS], BF16, tag="sdtu")
                        nc.vector.tensor_mul(dtu[:, :F], dt_t[g][:, s0:s0 + F],
                                             u_t[g][:, s0:s0 + F])
                        for n in range(NST):
                            nc.scalar.activation(dA_t[:, n * F:(n + 1) * F],
                                                 dt_t[g][:, s0:s0 + F], Act.Exp,
                                                 scale=vcol(g, n))
                        for n0, wid, eng in ((0, 4, nc.vector), (4, 4, nc.vector),
                                             (8, 4, nc.gpsimd), (12, 4, nc.gpsimd)):
                            eng.tensor_mul(
                                dBu_t[:, n0 * F:(n0 + wid) * F].rearrange(
                                    "p (a b) -> p a b", a=wid),
                                dtu[:, :F].unsqueeze(1).broadcast_to([128, wid, F]),
                                repB[:, n0 * F:(n0 + wid) * F].rearrange(
                                    "p (a b) -> p a b", a=wid))
                        for n in range(NST):
                            init = 0.0 if s == 0 else carry[:, g * NST + n:g * NST + n + 1]
                            nc.vector.tensor_tensor_scan(
                                dBu_t[:, n * F:(n + 1) * F],
                                dA_t[:, n * F:(n + 1) * F],
                                dBu_t[:, n * F:(n + 1) * F],
                                init, op0=Alu.mult, op1=Alu.add)
                        if s + 1 < nchunks:
                            nc.vector.tensor_copy(carry[:, g * NST:(g + 1) * NST],
                                                  dBu_t[:, F - 1:NST * F:F])
                        for n0, wid, eng in ((0, 4, nc.gpsimd), (4, 4, nc.gpsimd),
                                             (8, 4, nc.vector), (12, 4, nc.vector)):
                            eng.tensor_mul(dBu_t[:, n0 * F:(n0 + wid) * F],
                                           dBu_t[:, n0 * F:(n0 + wid) * F],
                                           repC[:, n0 * F:(n0 + wid) * F])
                        # y[t] = sum_n h*C : accumulate slots on PE via
                        # identity matmuls into one psum tile; consume the
                        # DVE-produced slots (8..15) first, Pool's (0..7) last.
                        # The psum tile is handed straight to stage O (the
                        # u*D+y scalar_tensor_tensor reads PSUM), no copy.
                        yps = yp.tile([128, TS], F32, tag="yps", name=f"yps{s}_{g}")
                        id_order = list(range(8, NST)) + list(range(8))
                        for j, n in enumerate(id_order):
                            nc.tensor.matmul(yps[:, :F], ident[:],
                                             dBu_t[:, n * F:(n + 1) * F],
                                             start=(j == 0), stop=(j == NST - 1))
                        ypss[(s, g)] = yps
                # ---- stage O ----
                for c0 in range(0, Lb, MM):
                    F = min(MM, Lb - c0)
                    ztmp = cw.tile([128, MM], F32, tag="ztmp", name="ztmp")
                    sden = cw.tile([128, MM], F32, tag="sden", name="sden")
                    for g in range(2):
                        nc.vector.scalar_tensor_tensor(
                            y_t[g][:, c0:c0 + F], u_t[g][:, c0:c0 + F], vcol(g, NST),
                            ypss[(c0 // TS, g)][:, :F], op0=Alu.mult, op1=Alu.add)
                        ps = mmp.tile([128, MM], F32, tag="mmps")
                        nc.tensor.matmul(ps[:, :F], winTb[:, (2 + g) * 128:(3 + g) * 128],
                                         x_ap[:, c0:c0 + F], start=True, stop=True)
                        # y *= z * sigmoid(z) ; sigmoid via exp table
                        nc.scalar.activation(sden[:, :F], ps[:, :F], Act.Exp,
                                             scale=-1.0)
                        nc.vector.tensor_scalar_add(sden[:, :F], sden[:, :F], 1.0)
                        nc.vector.reciprocal_approx_fast(sden[:, :F], sden[:, :F])
                        nc.vector.tensor_mul(y_t[g][:, c0:c0 + F], y_t[g][:, c0:c0 + F],
                                             sden[:, :F])
                        nc.scalar.activation(ztmp[:, :F], ps[:, :F], Act.Copy)
                        nc.vector.tensor_mul(y_t[g][:, c0:c0 + F], y_t[g][:, c0:c0 + F],
                                             ztmp[:, :F])
                    ps = mmp.tile([128, MM], F32, tag="mmps")
                    for g in range(2):
                        nc.tensor.matmul(ps[:, :F], woutTt[i][:, g * C:(g + 1) * C],
                                         y_t[g][:, c0:c0 + F], start=(g == 0), stop=(g == 1))
                    nc.scalar.activation(out_ap[:, c0:c0 + F], ps[:, :F], Act.Copy)
                    if out_dma is not None:
                        nc.sync.dma_start(out_dma[:, c0:c0 + F], out_ap[:, c0:c0 + F])

            def downconv(xt, off, j, Lb, out_ap):
                """xt: level tile; data at cols [off, off+Lb); front pad col off-1."""
                Lo = Lb // 2
                for c0 in range(0, Lo, MM):
                    F = min(MM, Lo - c0)
                    ps = mmp.tile([128, MM], F32, tag="mmps")
                    for k in range(3):
                        a = off + 2 * c0 + k - 1
                        nc.tensor.matmul(ps[:, :F], dcwTt[j][:, k * 128:(k + 1) * 128],
                                         xt[:, a:a + 2 * F - 1:2],
                                         start=(k == 0), stop=(k == 2))
                    nc.scalar.activation(out_ap[:, c0:c0 + F], ps[:, :F], Act.Identity,
                                         bias=gvecst[j][:, 0:1])

            def gate(t1_ap, t2_ap, j, Lb, f_ap):
                for c0 in range(0, Lb, MM):   # output chunk
                    F = min(MM, Lb - c0)
                    ch = c0 // 2
                    Fi = F // 2
                    t2u = gw.tile([128, MM], F32, tag="t2u")
                    pse = mmp.tile([128, MM], F32, tag="mmps")
                    nc.tensor.matmul(pse[:, :Fi], upwt[j][:, :128],
                                     t2_ap[:, ch:ch + Fi], start=True, stop=True)
                    nc.scalar.activation(t2u[:, 0:F:2], pse[:, :Fi], Act.Identity,
                                         bias=gvecst[j][:, 1:2])
                    pso = mmp.tile([128, MM], F32, tag="mmps")
                    nc.tensor.matmul(pso[:, :Fi], upwt[j][:, 128:],
                                     t2_ap[:, ch:ch + Fi], start=True, stop=True)
                    nc.scalar.activation(t2u[:, 1:F:2], pso[:, :Fi], Act.Identity,
                                         bias=gvecst[j][:, 1:2])
                    ps = mmp.tile([128, MM], F32, tag="mmps")
                    nc.tensor.matmul(ps[:, :F], wgTt[j][:, :128], t1_ap[:, c0:c0 + F],
                                     start=True, stop=False)
                    nc.tensor.matmul(ps[:, :F], wgTt[j][:, 128:], t2u[:, :F],
                                     start=False, stop=True)
                    wloc = gw.tile([128, MM], F32, tag="wloc")
                    # w = sigmoid(ps + b) = 1/(1+exp(-(ps+b))) via exp table
                    nc.scalar.activation(wloc[:, :F], ps[:, :F], Act.Exp,
                                         bias=gvecst[j][:, 4:5], scale=-1.0)
                    nc.vector.tensor_scalar_add(wloc[:, :F], wloc[:, :F], 1.0)
                    nc.vector.reciprocal_approx_fast(wloc[:, :F], wloc[:, :F])
                    m1 = gw.tile([128, MM], F32, tag="m1")
                    nc.vector.tensor_mul(m1[:, :F], t1_ap[:, c0:c0 + F], wloc[:, :F])
                    # m2 = t2u*(1-w), built in place: wloc <- t2u*w ; t2u <- t2u-wloc
                    nc.gpsimd.tensor_mul(wloc[:, :F], t2u[:, :F], wloc[:, :F])
                    nc.vector.tensor_sub(t2u[:, :F], t2u[:, :F], wloc[:, :F])
                    ps2 = mmp.tile([128, MM], F32, tag="mmps")
                    nc.tensor.matmul(ps2[:, :F], dbTt[j][:, :128], m1[:, :F],
                                     start=True, stop=False)
                    nc.tensor.matmul(ps2[:, :F], dbTt[j][:, 128:], t2u[:, :F],
                                     start=False, stop=True)
                    nc.scalar.activation(f_ap[:, c0:c0 + F], ps2[:, :F], Act.Identity,
                                         bias=gvecst[j][:, 3:4])

            # ---------- network ----------
            x1 = lvl.tile([128, 1025], F32, tag="x1")
            x2 = lvl.tile([128, 513], F32, tag="x2")
            x3 = lvl.tile([128, 257], F32, tag="x3")
            x4 = lvl.tile([128, 128], F32, tag="x4")
            e1 = lvl.tile([128, 1024], F32, tag="e1")
            e2 = lvl.tile([128, 512], F32, tag="e2")
            e3 = lvl.tile([128, 256], F32, tag="e3")
            e4 = lvl.tile([128, 128], F32, tag="e4")
            d4 = lvl.tile([128, 256], F32, tag="x3", name="d4")
            d3 = lvl.tile([128, 512], F32, tag="x2", name="d3")
            fbuf = lvl.tile([128, 1024], F32, tag="fbuf")

            nc.vector.memset(xi[0][:, :3], 0.0)
            nc.vector.memset(xi[1][:, :3], 0.0)
            nc.vector.memset(x1[:, 0:1], 0.0)
            nc.vector.memset(x2[:, 0:1], 0.0)
            nc.vector.memset(x3[:, 0:1], 0.0)
            nc.sync.dma_start(x1[:, 1:1025], xT_d[:, :])

            mamba(x1[:, 1:1025], 0, 1024, e1[:, :])
            downconv(x1, 1, 0, 1024, x2[:, 1:513])
            mamba(x2[:, 1:513], 1, 512, e2[:, :])
            downconv(x2, 1, 1, 512, x3[:, 1:257])
            mamba(x3[:, 1:257], 2, 256, e3[:, :])
            downconv(x3, 1, 2, 256, x4[:, :])
            mamba(x4[:, :], 3, 128, e4[:, :])
            gate(e3[:, :], e4[:, :], 0, 256, fbuf[:, :256])
            mamba(fbuf[:, :256], 4, 256, d4[:, :])
            gate(e2[:, :], d4[:, :], 1, 512, fbuf[:, :512])
            mamba(fbuf[:, :512], 5, 512, d3[:, :])
            gate(e1[:, :], d3[:, :], 2, 1024, fbuf[:, :])
            d2 = x1  # x1 dead by now; reuse its slot
            mamba(fbuf[:, :], 6, 1024, d2[:, 1:1025], out_dma=out_d)

    bacc.get_activation_tables = _patched_gat
    try:
        nc.compile()
    finally:
        bacc.get_activation_tables = _orig_gat
    return nc


def _get_program():
    if "nc" not in _CACHE:
        _CACHE["nc"] = _build()
    return _CACHE["nc"]


def kernel(**inputs):
    from concourse.bass_utils import run_bass_kernel_spmd

    nc = _get_program()
    w = _prep_weights(inputs)
    x = np.asarray(inputs["x"], np.float32)  # [B, L, C]
    in_maps = []
    for c in range(NCORES):
        m = {"xT": np.ascontiguousarray(x[c % B].T)}
        m.update(w)
        in_maps.append(m)
    res = run_bass_kernel_spmd(nc, in_maps, list(range(NCORES)))
    out = np.empty((B, L0, C), np.float32)
    for b in range(B):
        out[b] = res.results[b]["out"].T
    return out


# revision 20
# speedup vs baseline: 1.0007x; 1.0007x over previous
"""Trainium2 Bass kernel for the Mamba U-Net model (nn_Model_20770461843918).

Batch-data-parallel SPMD over 8 NeuronCores (4 batch elements; cores c and
c+4 duplicate work, outputs read from cores 0-3).  Per core the whole
7-block Mamba U-Net runs locally with partitions = inner channel d:
  PE  : all matmuls (in/x/dt/out projections, depthwise conv via diagonal
        matmuls, down/up/gate convs), plus the n-state reduction
        y = sum_n h*C as accumulating identity matmuls into PSUM
  DMA : per-timestep B/C rows broadcast across the 128 partitions straight
        from the DRAM bounce buffer (partition-stride-0 descriptors)
  ACT : exp(dt*A) per state n, exp/log for softplus and for sigmoid
        (sigmoid = 1/(1+exp(-x)) so every activation stays in the single
        natural_log_exp act table -> one table load total)
  DVE : selective scan via tensor_tensor_scan (internal fp32 state, bf16
        operands/output), fast reciprocal, share of elementwise muls
  Pool: remaining share of the dBu/h*C elementwise muls
Precision split: the U-Net trunk (level tensors, in/out projections, gates,
downconvs) stays fp32; the scan-stage inner pipeline (conv->u, x-proj, dt,
dBu/h/h*C cubes, B/C reps) rides in bf16 with fp32 scan state and PSUM.
"""
import numpy as np
import ml_dtypes

B, L0, C = 4, 1024, 128
DI, NST, R, KC = 256, 16, 8, 4
NV = NST + 4          # packed per-partition vec cols: A[16], D, convb, bdt, -convb
NCORES = 8
TS = 512              # scan-stage time chunk
MM = 512              # matmul-stage time chunk


# bf16 panel pack layout (cols): identity only
TOTW16 = 128
# f32 panel pack layout (cols): per block wxT(2x64)+woutT (2x128), per gate
# dcwT(384)+upw(256)+wgT(256)+dbT(256), then 7x2xNV vec cols + 3x5 gate vecs
BLKW32 = 128 + 256
GATW32 = 384 + 256 + 256 + 256
TOTW32 = 7 * BLKW32 + 3 * GATW32 + 7 * 2 * NV + 3 * 5

_CACHE = {}


def _prep_weights(inp):
    f32 = np.float32
    bf16 = ml_dtypes.bfloat16
    g = lambda k: np.asarray(inp[k], f32)
    m_Win, m_convw, m_convb = g("m_Win"), g("m_convw"), g("m_convb")
    m_Wx, m_Wdt, m_bdt = g("m_Wx"), g("m_Wdt"), g("m_bdt")
    m_Alog, m_D, m_Wout = g("m_Alog"), g("m_D"), g("m_Wout")
    dc_w, dc_b = g("dc_w"), g("dc_b")
    wg_W, wg_b, db_W, db_b = g("wg_W"), g("wg_b"), g("db_W"), g("db_b")
    up_w, up_b = g("up_w"), g("up_b")

    winT = np.ascontiguousarray(m_Win.transpose(0, 2, 1))                # [7, C, 512] f32
    cd = np.zeros((7, 2, KC, 128, 128), f32)
    idx = np.arange(128)
    for i in range(7):
        for gg in range(2):
            for k in range(KC):
                cd[i, gg, k, idx, idx] = m_convw[i, gg * 128:(gg + 1) * 128, k]
    # sbuf layout [128, (g, k, 128)]: partition = k_in, free-block (g,k) = lhsT
    convdiag = np.ascontiguousarray(
        cd.transpose(0, 1, 3, 2, 4)).reshape(7, 2, 128, KC * 128)
    wxT_raw = np.ascontiguousarray(m_Wx.transpose(0, 2, 1)).reshape(7, 2, 128, R + 2 * NST)
    wxT = np.zeros((7, 2, 128, 64), f32)
    wxT[..., :R] = wxT_raw[..., :R]          # dt rows -> psum partitions 0..7
    wxT[..., 32:64] = wxT_raw[..., R:]       # B/C rows -> psum partitions 32..63
    wdtT = np.ascontiguousarray(m_Wdt.transpose(0, 2, 1))                # [7, R, DI]
    wdtall = wdtT.transpose(1, 0, 2).reshape(R, 7 * DI).astype(bf16)
    A = -np.exp(m_Alog)                                                  # [7, DI, N]
    vec = np.zeros((7, 2, 128, NV), f32)
    for gg in range(2):
        sl = slice(gg * 128, (gg + 1) * 128)
        vec[:, gg, :, :NST] = A[:, sl, :]
        vec[:, gg, :, NST] = m_D[:, sl]
        vec[:, gg, :, NST + 1] = m_convb[:, sl]
        vec[:, gg, :, NST + 2] = m_bdt[:, sl]
        vec[:, gg, :, NST + 3] = -m_convb[:, sl]
    woutT = np.ascontiguousarray(m_Wout.transpose(0, 2, 1)).reshape(7, 2, 128, C)
    # dc_w[j, co, ci, k] -> [j, ci, (k, co)]
    dcwT = np.ascontiguousarray(dc_w.transpose(0, 2, 3, 1)).reshape(3, 128, 3 * 128)
    # up_w[j, ci, co, k] -> [j, ci, (k, co)]
    upw = np.ascontiguousarray(up_w.transpose(0, 1, 3, 2)).reshape(3, 128, 2 * 128)
    wgT = np.ascontiguousarray(wg_W.transpose(0, 2, 1)).reshape(3, 2, 128, 128)
    dbT = np.ascontiguousarray(db_W.transpose(0, 2, 1)).reshape(3, 2, 128, 128)
    gv = np.zeros((3, 128, 5), f32)
    gv[:, :, 0], gv[:, :, 1], gv[:, :, 2], gv[:, :, 3] = dc_b, up_b, wg_b, db_b
    gv[:, :, 4] = -wg_b
    # bf16 pack: identity only
    p16 = [np.eye(128, dtype=bf16)]
    # f32 pack: wxT, woutT, gate weights, vec cols
    p32 = []
    for i in range(7):
        p32 += [wxT[i, 0], wxT[i, 1], woutT[i, 0], woutT[i, 1]]
    for j in range(3):
        p32 += [dcwT[j], upw[j], wgT[j, 0], wgT[j, 1], dbT[j, 0], dbT[j, 1]]
    p32 += [vec[i, gg] for i in range(7) for gg in range(2)]
    p32 += [gv[j] for j in range(3)]
    return {"winT": winT, "convdiag": convdiag, "wdtall": wdtall,
            "wtpack16": np.ascontiguousarray(np.concatenate(p16, axis=1)),
            "wtpack32": np.ascontiguousarray(np.concatenate(p32, axis=1))}


def _build():
    import concourse.bacc as bacc
    import concourse.tile as tile
    import concourse.mybir as mybir
    from concourse.hw_specs import get_activation_tables as _gat

    F32 = mybir.dt.float32
    BF16 = mybir.dt.bfloat16
    Alu = mybir.AluOpType
    Act = mybir.ActivationFunctionType

    # Steer the act-table placement pass to the one table that holds every
    # function we use (Copy/Identity/Exp/Ln), so exactly one table load is
    # emitted.  Table ids stay aligned with act_info.json: we only hide our
    # functions from the OTHER tables during placement.
    _KEEP = "natural_log_exp_and_others"
    _OURS = {Act.Copy, Act.Identity, Act.Exp, Act.Ln}
    _orig_gat = bacc.get_activation_tables

    def _patched_gat(arch):
        tabs = _gat(arch)
        assert _KEEP in tabs and _OURS <= tabs[_KEEP]
        return {k: (v if k == _KEEP else set(v) - _OURS) for k, v in tabs.items()}

    nc = bacc.Bacc("TRN2", target_bir_lowering=False, debug=False,
                   num_devices=NCORES)

    xT_d = nc.declare_dram_parameter("xT", [C, L0], F32, isOutput=False)
    out_d = nc.declare_dram_parameter("out", [C, L0], F32, isOutput=True)
    dram = {}
    for name, shape, dt in [
        ("winT", [7, C, 2 * DI], F32),
        ("convdiag", [7, 2, 128, KC * 128], F32),
        ("wdtall", [R, 7 * DI], BF16),
        ("wtpack16", [128, TOTW16], BF16),
        ("wtpack32", [128, TOTW32], F32),
    ]:
        dram[name] = nc.declare_dram_parameter(name, shape, dt, isOutput=False)
    bc_dram4 = [[nc.dram_tensor(f"bc_bounce{p}_{s}", [2 * NST, TS], BF16)
                 for s in range(2)] for p in range(2)]

    with tile.TileContext(nc) as tc:
        with tc.tile_pool(name="wt", bufs=1) as wt, \
             tc.tile_pool(name="lvl", bufs=1) as lvl, \
             tc.tile_pool(name="blk", bufs=1) as blk, \
             tc.tile_pool(name="cube", bufs=2) as cube, \
             tc.tile_pool(name="repk", bufs=2) as repk, \
             tc.tile_pool(name="cw", bufs=2) as cw, \
             tc.tile_pool(name="ubuf", bufs=1) as ubuf, \
             tc.tile_pool(name="gw", bufs=1) as gw, \
             tc.tile_pool(name="cwc", bufs=2) as cwc, \
             tc.tile_pool(name="mmp", bufs=3, space="PSUM") as mmp, \
             tc.tile_pool(name="yp", bufs=4, space="PSUM") as yp, \
             tc.tile_pool(name="xdbp", bufs=1, space="PSUM") as xdbp:

            def load_blk(i):
                winTb = cw.tile([C, 2 * DI], F32, tag="winT", name=f"winTb{i}")
                nc.scalar.dma_start(winTb[:], dram["winT"][i])
                cdw = cwc.tile([128, 2 * KC * 128], F32, tag="convdiag",
                               name=f"cdw{i}")
                nc.scalar.dma_start(cdw[:, :KC * 128], dram["convdiag"][i, 0])
                nc.scalar.dma_start(cdw[:, KC * 128:], dram["convdiag"][i, 1])
                return cdw, winTb

            preload = {0: load_blk(0)}

            w16 = wt.tile([128, TOTW16], BF16, tag="w16")
            nc.scalar.dma_start(w16[:], dram["wtpack16"][:])
            w32 = wt.tile([128, TOTW32], F32, tag="w32")
            nc.scalar.dma_start(w32[:, :TOTW32 // 2], dram["wtpack32"][:, :TOTW32 // 2])
            nc.scalar.dma_start(w32[:, TOTW32 // 2:], dram["wtpack32"][:, TOTW32 // 2:])
            wdtall = wt.tile([R, 7 * DI], BF16, tag="wdtall")
            nc.scalar.dma_start(wdtall[:], dram["wdtall"][:])
            ident = w16[:, :]
            wdtTt = [wdtall[:, i * DI:(i + 1) * DI] for i in range(7)]
            wxTt = [w32[:, i * BLKW32:i * BLKW32 + 128] for i in range(7)]
            woutTt = [w32[:, i * BLKW32 + 128:(i + 1) * BLKW32] for i in range(7)]
            og = 7 * BLKW32
            dcwTt, upwt, wgTt, dbTt = [], [], [], []
            for j in range(3):
                o = og + j * GATW32
                dcwTt.append(w32[:, o:o + 384])
                upwt.append(w32[:, o + 384:o + 640])
                wgTt.append(w32[:, o + 640:o + 896])
                dbTt.append(w32[:, o + 896:o + 1152])
            ov = og + 3 * GATW32
            vecst = [w32[:, ov + i * 2 * NV:ov + (i + 1) * 2 * NV] for i in range(7)]
            ogv = ov + 14 * NV
            gvecst = [w32[:, ogv + j * 5:ogv + j * 5 + 5] for j in range(3)]

            # per-block working tiles (reused across blocks)
            xi = [blk.tile([128, L0 + 3], F32, tag=f"xi{g}", name=f"xi{g}")
                  for g in range(2)]
            y_t = [blk.tile([128, L0], F32, tag=f"y{g}", name=f"y{g}")
                   for g in range(2)]
            xdbR = blk.tile([R, L0], BF16, tag="xdbR")
            bc16 = blk.tile([2 * NST, L0], BF16, tag="bc16")
            carry = blk.tile([128, 2 * NST], BF16, tag="carry")

            def mamba(x_ap, i, Lb, out_ap, out_dma=None):
                cdw, winTb = preload.pop(i) if i in preload else load_blk(i)
                u_t = [ubuf.tile([128, L0], F32, tag=f"u{g}", name=f"u{g}_{i}")
                       for g in range(2)]
                dt_t = [ubuf.tile([128, L0], BF16, tag=f"dt{g}", name=f"dt{g}_{i}")
                        for g in range(2)]
                vecs = vecst[i]
                ypss = {}

                def vcol(g, c):
                    return vecs[:, g * NV + c: g * NV + c + 1]
                # ---- stage M ----
                for c0 in range(0, Lb, MM):
                    F = min(MM, Lb - c0)
                    ztmp = cw.tile([128, MM], F32, tag="ztmp", name="ztmpM")
                    for p in range(2):
                        ps = mmp.tile([128, MM], F32, tag="mmps")
                        nc.tensor.matmul(ps[:, :F], winTb[:, p * 128:(p + 1) * 128],
                                         x_ap[:, c0:c0 + F], start=True, stop=True)
                        nc.scalar.activation(xi[p][:, 3 + c0:3 + c0 + F], ps[:, :F], Act.Copy)
                    for g in range(2):
                        ps = mmp.tile([128, MM], F32, tag="mmps")
                        for k in range(KC):
                            nc.tensor.matmul(
                                ps[:, :F],
                                cdw[:, (g * KC + k) * 128:(g * KC + k + 1) * 128],
                                xi[g][:, c0 + k:c0 + k + F],
                                start=(k == 0), stop=(k == KC - 1))
                        nc.scalar.activation(u_t[g][:, c0:c0 + F], ps[:, :F], Act.Identity,
                                             bias=vcol(g, NST + 1))
                        # sigmoid via exp (stay in the exp/ln act table):
                        # e = exp(-(x+b)); u = (x+b) / (1+e)
                        nc.scalar.activation(ztmp[:, :F], ps[:, :F], Act.Exp,
                                             bias=vcol(g, NST + 3), scale=-1.0)
                        nc.vector.tensor_scalar_add(ztmp[:, :F], ztmp[:, :F], 1.0)
                        nc.vector.reciprocal_approx_fast(ztmp[:, :F], ztmp[:, :F])
                        nc.vector.tensor_mul(u_t[g][:, c0:c0 + F], u_t[g][:, c0:c0 + F],
                                             ztmp[:, :F])
                    psx = xdbp.tile([64, MM], F32, tag="xdbps")
                    for g in range(2):
                        nc.tensor.matmul(psx[:, :F],
                                         wxTt[i][:, g * 64:(g + 1) * 64],
                                         u_t[g][:, c0:c0 + F], start=(g == 0), stop=(g == 1))
                    nc.scalar.activation(xdbR[:, c0:c0 + F], psx[:R, :F], Act.Copy)
                    nc.scalar.activation(bc16[:, c0:c0 + F], psx[32:, :F], Act.Copy)
                    nc.sync.dma_start(bc_dram4[i % 2][c0 // TS][:, :F],
                                      bc16[:, c0:c0 + F])
                    for g in range(2):
                        ps = mmp.tile([128, MM], F32, tag="mmps")
                        nc.tensor.matmul(ps[:, :F], wdtTt[i][:, g * 128:(g + 1) * 128],
                                         xdbR[:, c0:c0 + F], start=True, stop=True)
                        nc.scalar.activation(ztmp[:, :F], ps[:, :F], Act.Exp,
                                             bias=vcol(g, NST + 2))
                        nc.scalar.activation(dt_t[g][:, c0:c0 + F], ztmp[:, :F], Act.Ln,
                                             bias=1.0)
                # ---- stage S ----
                nchunks = (Lb + TS - 1) // TS
                for s in range(nchunks):
                    s0 = s * TS
                    F = min(TS, Lb - s0)
                    bc_dram = bc_dram4[i % 2][s]
                    # B/C rows broadcast across partitions straight from DRAM
                    repB = repk.tile([128, NST * TS], BF16, tag="rep", name="repB")
                    repC = repk.tile([128, NST * TS], BF16, tag="rep", name="repC")
                    for hq in range(2):
                        nh = 8 * hq
                        nc.sync.dma_start(
                            repB[:, nh * F:(nh + 8) * F].rearrange(
                                "p (n t) -> p n t", n=8),
                            bc_dram[nh:nh + 8, :F].unsqueeze(0).broadcast_to(
                                [128, 8, F]))
                    nc.sync.dma_start(
                        repC[:, :NST * F].rearrange("p (n t) -> p n t", n=NST),
                        bc_dram[NST:, :F].unsqueeze(0).broadcast_to([128, NST, F]))
                    for g in range(2):
                        dA_t = cube.tile([128, NST * TS], BF16, tag="dA")
                        dBu_t = cube.tile([128, NST * TS], BF16, tag="dBu")
                        dtu = cw.tile([128, TS], BF16, tag="sdtu")
                        nc.vector.tensor_mul(dtu[:, :F], dt_t[g][:, s0:s0 + F],
                                             u_t[g][:, s0:s0 + F])
                        for n in range(NST):
                            nc.scalar.activation(dA_t[:, n * F:(n + 1) * F],
                                                 dt_t[g][:, s0:s0 + F], Act.Exp,
                                                 scale=vcol(g, n))
                        for n0, wid, eng in ((0, 4, nc.vector), (4, 4, nc.vector),
                                             (8, 4, nc.gpsimd), (12, 4, nc.gpsimd)):
                            eng.tensor_mul(
                                dBu_t[:, n0 * F:(n0 + wid) * F].rearrange(
                                    "p (a b) -> p a b", a=wid),
                                dtu[:, :F].unsqueeze(1).broadcast_to([128, wid, F]),
                                repB[:, n0 * F:(n0 + wid) * F].rearrange(
                                    "p (a b) -> p a b", a=wid))
                        for n in range(NST):
                            init = 0.0 if s == 0 else carry[:, g * NST + n:g * NST + n + 1]
                            nc.vector.tensor_tensor_scan(
                                dBu_t[:, n * F:(n + 1) * F],
                                dA_t[:, n * F:(n + 1) * F],
                                dBu_t[:, n * F:(n + 1) * F],
                                init, op0=Alu.mult, op1=Alu.add)
                        if s + 1 < nchunks:
                            nc.vector.tensor_copy(carry[:, g * NST:(g + 1) * NST],
                                                  dBu_t[:, F - 1:NST * F:F])
                        for n0, wid, eng in ((0, 4, nc.gpsimd), (4, 4, nc.gpsimd),
                                             (8, 4, nc.vector), (12, 4, nc.vector)):
                            eng.tensor_mul(dBu_t[:, n0 * F:(n0 + wid) * F],
                                           dBu_t[:, n0 * F:(n0 + wid) * F],
                                           repC[:, n0 * F:(n0 + wid) * F])
                        # y[t] = sum_n h*C : accumulate slots on PE via
                        # identity matmuls into one psum tile; consume the
                        # DVE-produced slots (8..15) first, Pool's (0..7) last.
                        # The psum tile is handed straight to stage O (the
                        # u*D+y scalar_tensor_tensor reads PSUM), no copy.
                        yps = yp.tile([128, TS], F32, tag="yps", name=f"yps{s}_{g}")
                        id_order = list(range(8, NST)) + list(range(8))
                        for j, n in enumerate(id_order):
                            nc.tensor.matmul(yps[:, :F], ident[:],
                                             dBu_t[:, n * F:(n + 1) * F],
                                             start=(j == 0), stop=(j == NST - 1))
                        ypss[(s, g)] = yps
                # ---- stage O ----
                for c0 in range(0, Lb, MM):
                    F = min(MM, Lb - c0)
                    ztmp = cw.tile([128, MM], F32, tag="ztmp", name="ztmp")
                    sden = cw.tile([128, MM], F32, tag="sden", name="sden")
                    for g in range(2):
                        nc.vector.scalar_tensor_tensor(
                            y_t[g][:, c0:c0 + F], u_t[g][:, c0:c0 + F], vcol(g, NST),
                            ypss[(c0 // TS, g)][:, :F], op0=Alu.mult, op1=Alu.add)
                        ps = mmp.tile([128, MM], F32, tag="mmps")
                        nc.tensor.matmul(ps[:, :F], winTb[:, (2 + g) * 128:(3 + g) * 128],
                                         x_ap[:, c0:c0 + F], start=True, stop=True)
                        # y *= z * sigmoid(z) ; sigmoid via exp table
                        nc.scalar.activation(sden[:, :F], ps[:, :F], Act.Exp,
                                             scale=-1.0)
                        nc.vector.tensor_scalar_add(sden[:, :F], sden[:, :F], 1.0)
                        nc.vector.reciprocal_approx_fast(sden[:, :F], sden[:, :F])
                        nc.vector.tensor_mul(y_t[g][:, c0:c0 + F], y_t[g][:, c0:c0 + F],
                                             sden[:, :F])
                        nc.scalar.activation(ztmp[:, :F], ps[:, :F], Act.Copy)
                        nc.vector.tensor_mul(y_t[g][:, c0:c0 + F], y_t[g][:, c0:c0 + F],
                                             ztmp[:, :F])
                    ps = mmp.tile([128, MM], F32, tag="mmps")
                    for g in range(2):
                        nc.tensor.matmul(ps[:, :F], woutTt[i][:, g * C:(g + 1) * C],
                                         y_t[g][:, c0:c0 + F], start=(g == 0), stop=(g == 1))
                    nc.scalar.activation(out_ap[:, c0:c0 + F], ps[:, :F], Act.Copy)
                    if out_dma is not None:
                        nc.sync.dma_start(out_dma[:, c0:c0 + F], out_ap[:, c0:c0 + F])

            def downconv(xt, off, j, Lb, out_ap):
                """xt: level tile; data at cols [off, off+Lb); front pad col off-1."""
                Lo = Lb // 2
                for c0 in range(0, Lo, MM):
                    F = min(MM, Lo - c0)
                    ps = mmp.tile([128, MM], F32, tag="mmps")
                    for k in range(3):
                        a = off + 2 * c0 + k - 1
                        nc.tensor.matmul(ps[:, :F], dcwTt[j][:, k * 128:(k + 1) * 128],
                                         xt[:, a:a + 2 * F - 1:2],
                                         start=(k == 0), stop=(k == 2))
                    nc.scalar.activation(out_ap[:, c0:c0 + F], ps[:, :F], Act.Identity,
                                         bias=gvecst[j][:, 0:1])

            def gate(t1_ap, t2_ap, j, Lb, f_ap):
                for c0 in range(0, Lb, MM):   # output chunk
                    F = min(MM, Lb - c0)
                    ch = c0 // 2
                    Fi = F // 2
                    t2u = gw.tile([128, MM], F32, tag="t2u")
                    pse = mmp.tile([128, MM], F32, tag="mmps")
                    nc.tensor.matmul(pse[:, :Fi], upwt[j][:, :128],
                                     t2_ap[:, ch:ch + Fi], start=True, stop=True)
                    nc.scalar.activation(t2u[:, 0:F:2], pse[:, :Fi], Act.Identity,
                                         bias=gvecst[j][:, 1:2])
                    pso = mmp.tile([128, MM], F32, tag="mmps")
                    nc.tensor.matmul(pso[:, :Fi], upwt[j][:, 128:],
                                     t2_ap[:, ch:ch + Fi], start=True, stop=True)
                    nc.scalar.activation(t2u[:, 1:F:2], pso[:, :Fi], Act.Identity,
                                         bias=gvecst[j][:, 1:2])
                    ps = mmp.tile([128, MM], F32, tag="mmps")
                    nc.tensor.matmul(ps[:, :F], wgTt[j][:, :128], t1_ap[:, c0:c0 + F],
                                     start=True, stop=False)
                    nc.tensor.matmul(ps[:, :F], wgTt[j][:, 128:], t2u[:, :F],
                                     start=False, stop=True)
                    wloc = gw.tile([128, MM], F32, tag="wloc")
                    # w = sigmoid(ps + b) = 1/(1+exp(-(ps+b))) via exp table
                    nc.scalar.activation(wloc[:, :F], ps[:, :F], Act.Exp,
                                         bias=gvecst[j][:, 4:5], scale=-1.0)
                    nc.vector.tensor_scalar_add(wloc[:, :F], wloc[:, :F], 1.0)
                    nc.vector.reciprocal_approx_fast(wloc[:, :F], wloc[:, :F])
                    m1 = gw.tile([128, MM], F32, tag="m1")
                    nc.vector.tensor_mul(m1[:, :F], t1_ap[:, c0:c0 + F], wloc[:, :F])
                    # m2 = t2u*(1-w), built in place: wloc <- t2u*w ; t2u <- t2u-wloc
                    nc.gpsimd.tensor_mul(wloc[:, :F], t2u[:, :F], wloc[:, :F])
                    nc.vector.tensor_sub(t2u[:, :F], t2u[:, :F], wloc[:, :F])
                    ps2 = mmp.tile([128, MM], F32, tag="mmps")
                    nc.tensor.matmul(ps2[:, :F], dbTt[j][:, :128], m1[:, :F],
                                     start=True, stop=False)
                    nc.tensor.matmul(ps2[:, :F], dbTt[j][:, 128:], t2u[:, :F],
                                     start=False, stop=True)
                    nc.scalar.activation(f_ap[:, c0:c0 + F], ps2[:, :F], Act.Identity,
                                         bias=gvecst[j][:, 3:4])

            # ---------- network ----------
            x1 = lvl.tile([128, 1025], F32, tag="x1")
            x2 = lvl.tile([128, 513], F32, tag="x2")
            x3 = lvl.tile([128, 257], F32, tag="x3")
            x4 = lvl.tile([128, 128], F32, tag="x4")
            e1 = lvl.tile([128, 1024], F32, tag="e1")
            e2 = lvl.tile([128, 512], F32, tag="e2")
            e3 = lvl.tile([128, 256], F32, tag="e3")
            e4 = lvl.tile([128, 128], F32, tag="e4")
            d4 = lvl.tile([128, 256], F32, tag="x3", name="d4")
            d3 = lvl.tile([128, 512], F32, tag="x2", name="d3")
            fbuf = lvl.tile([128, 1024], F32, tag="fbuf")

            nc.vector.memset(xi[0][:, :3], 0.0)
            nc.vector.memset(xi[1][:, :3], 0.0)
            nc.vector.memset(x1[:, 0:1], 0.0)
            nc.vector.memset(x2[:, 0:1], 0.0)
            nc.vector.memset(x3[:, 0:1], 0.0)
            nc.sync.dma_start(x1[:, 1:1025], xT_d[:, :])

            mamba(x1[:, 1:1025], 0, 1024, e1[:, :])
            # the downconv chain needs only x1: emit it as one run so the
            # PE work overlaps the later mamba blocks' scan phases
            downconv(x1, 1, 0, 1024, x2[:, 1:513])
            downconv(x2, 1, 1, 512, x3[:, 1:257])
            downconv(x3, 1, 2, 256, x4[:, :])
            mamba(x2[:, 1:513], 1, 512, e2[:, :])
            mamba(x3[:, 1:257], 2, 256, e3[:, :])
            mamba(x4[:, :], 3, 128, e4[:, :])
            gate(e3[:, :], e4[:, :], 0, 256, fbuf[:, :256])
            mamba(fbuf[:, :256], 4, 256, d4[:, :])
            gate(e2[:, :], d4[:, :], 1, 512, fbuf[:, :512])
            mamba(fbuf[:, :512], 5, 512, d3[:, :])
            gate(e1[:, :], d3[:, :], 2, 1024, fbuf[:, :])
            d2 = x1  # x1 dead by now; reuse its slot
            mamba(fbuf[:, :], 6, 1024, d2[:, 1:1025], out_dma=out_d)

    bacc.get_activation_tables = _patched_gat
    try:
        nc.compile()
    finally:
        bacc.get_activation_tables = _orig_gat
    return nc


def _get_program():
    if "nc" not in _CACHE:
        _CACHE["nc"] = _build()
    return _CACHE["nc"]


def kernel(**inputs):
    from concourse.bass_utils import run_bass_kernel_spmd

    nc = _get_program()
    w = _prep_weights(inputs)
    x = np.asarray(inputs["x"], np.float32)  # [B, L, C]
    in_maps = []
    for c in range(NCORES):
        m = {"xT": np.ascontiguousarray(x[c % B].T)}
        m.update(w)
        in_maps.append(m)
    res = run_bass_kernel_spmd(nc, in_maps, list(range(NCORES)))
    out = np.empty((B, L0, C), np.float32)
    for b in range(B):
        out[b] = res.results[b]["out"].T
    return out
